# revision 1
# baseline (speedup 1.0000x reference)
"""Trainium2 Bass kernel for nn_ArmInt_19911468384433 (dense_mlp, 8 cores).

Data-parallel: x [2097152, 32] is sharded by rows across 8 NeuronCores;
the tiny 32x32 weights are folded/replicated. All math (3 integer-MLP
layers with emulated fixed-point rounding, exp/clip head) runs on device.
Host does only layout permutation (shard + transpose-pack of the input,
inverse reshape of the output) and weight folding.

Device algorithm per core (S = 262144 rows = 128 tiles of 2048 rows):
  x is pre-packed on host as xd[t, 32b+c, f'] = x[2048 t + 512 b + f', c]
  (4 row-groups stacked on partitions, channels-major: matmul-ready).
  Per tile:
    mm1 : ps1 = W1s.T @ xd[t]     W1s = block-diag4(w0.T + 256 I), f32
    ep1 : t1 = Relu(ps1 + bc1)  (ACT, per-partition bias b0/256 + 2^-9)
          h1 = rne(t1) via fused (+2^23, -2^23) tensor_scalar -> fp16
    mm2 : ps2 = W2s.T @ h1        W2s = block-diag4((w1.T + 256 I)/256), fp16
    ep2 : same -> h2 fp16
    mm3 : ps3pack += W3s_tau.T @ h2   (16 tiles accumulate into one PSUM
          bank; W3s_tau nonzero only in its 8 output rows m = 8 tau+4 o+b)
  Per 16-tile pack:
    t3 = ps3pack + bc3 (ACT Identity, bias b_out/256 + 2^-9)
    y  = rne(t3)                   (integer y = 256 * raw)
    outA = y/256                   (mu and log_scale planes)
    e = Exp(y/256 - 4); outB = clip(e, e^-4.6, e^5)

Exactness: layers are integer-valued; rne(v + 2^-9) == trunc-round-half-
away-from-zero for the 1/256-granular values here (verified vs reference,
rel err ~5e-3 from fp32 accumulation-order only).
"""
import sys

sys.path.insert(0, "/opt/trn_rl_repo")

from contextlib import ExitStack

import numpy as np

import concourse.bacc as bacc
import concourse.bass as bass
import concourse.tile as tile
from concourse import mybir
from concourse.bass_utils import run_bass_kernel_spmd

F32 = mybir.dt.float32
F16 = mybir.dt.float16
AF = mybir.ActivationFunctionType
ALU = mybir.AluOpType

B = 2097152
C = 32
NCORES = 8
S = B // NCORES            # 262144 rows per core
NT = S // 2048             # 128 tiles per core
NPACK = NT // 16           # 8 packs per core
C2 = float(2.0 ** 23)
CTIE = float(2.0 ** -9)

_compiled = {}


def _build_graph():
    nc = bacc.Bacc("TRN2", target_bir_lowering=False, debug=False)
    xd = nc.declare_dram_parameter("xd", [NT, 128, 512], F32, isOutput=False)
    w1s = nc.declare_dram_parameter("w1s", [128, 128], F32, isOutput=False)
    w2s = nc.declare_dram_parameter("w2s", [128, 128], F16, isOutput=False)
    w3s = nc.declare_dram_parameter("w3s", [128, 2048], F16, isOutput=False)
    bc1 = nc.declare_dram_parameter("bc1", [128, 1], F32, isOutput=False)
    bc2 = nc.declare_dram_parameter("bc2", [128, 1], F32, isOutput=False)
    bc3 = nc.declare_dram_parameter("bc3", [128, 1], F32, isOutput=False)
    outa = nc.declare_dram_parameter("outa", [NPACK, 128, 512], F32, isOutput=True)
    outb = nc.declare_dram_parameter("outb", [NPACK, 128, 512], F32, isOutput=True)

    with ExitStack() as ctx:
        tc = ctx.enter_context(tile.TileContext(nc))
        consts = ctx.enter_context(tc.tile_pool(name="consts", bufs=1))
        xpool = ctx.enter_context(tc.tile_pool(name="xpool", bufs=3))
        tpool = ctx.enter_context(tc.tile_pool(name="tpool", bufs=3))
        hpool = ctx.enter_context(tc.tile_pool(name="hpool", bufs=3))
        opool = ctx.enter_context(tc.tile_pool(name="opool", bufs=2))
        ps1p = ctx.enter_context(tc.tile_pool(name="ps1p", bufs=2, space="PSUM"))
        ps2p = ctx.enter_context(tc.tile_pool(name="ps2p", bufs=2, space="PSUM"))
        ps3p = ctx.enter_context(tc.tile_pool(name="ps3p", bufs=2, space="PSUM"))

        w1_sb = consts.tile([128, 128], F32, tag="w1")
        nc.gpsimd.dma_start(out=w1_sb, in_=w1s[:])
        w2_sb = consts.tile([128, 128], F16, tag="w2")
        nc.gpsimd.dma_start(out=w2_sb, in_=w2s[:])
        w3_sb = consts.tile([128, 2048], F16, tag="w3")
        nc.gpsimd.dma_start(out=w3_sb, in_=w3s[:])
        bc1_sb = consts.tile([128, 1], F32, tag="bc1")
        nc.gpsimd.dma_start(out=bc1_sb, in_=bc1[:])
        bc2_sb = consts.tile([128, 1], F32, tag="bc2")
        nc.gpsimd.dma_start(out=bc2_sb, in_=bc2[:])
        bc3_sb = consts.tile([128, 1], F32, tag="bc3")
        nc.gpsimd.dma_start(out=bc3_sb, in_=bc3[:])
        bm4_sb = consts.tile([128, 1], F32, tag="bm4")
        nc.vector.memset(bm4_sb, -4.0)

        for pack in range(NPACK):
            ps3 = ps3p.tile([128, 512], F32, tag="ps3")
            for tau in range(16):
                t = pack * 16 + tau
                xin = xpool.tile([128, 512], F32, tag="xin")
                nc.sync.dma_start(out=xin, in_=xd[t])

                ps1 = ps1p.tile([128, 512], F32, tag="ps1")
                nc.tensor.matmul(ps1, w1_sb, xin, start=True, stop=True)
                t1 = tpool.tile([128, 512], F32, tag="t1")
                nc.scalar.activation(t1, ps1, AF.Relu, bias=bc1_sb, scale=1.0)
                h1 = hpool.tile([128, 512], F16, tag="h1")
                eng1 = nc.vector if (t % 2 == 0) else nc.gpsimd
                eng1.tensor_scalar(h1, t1, C2, C2, ALU.add, ALU.subtract)

                ps2 = ps2p.tile([128, 512], F32, tag="ps2")
                nc.tensor.matmul(ps2, w2_sb, h1, start=True, stop=True)
                t2 = tpool.tile([128, 512], F32, tag="t2")
                nc.scalar.activation(t2, ps2, AF.Relu, bias=bc2_sb, scale=1.0)
                h2 = hpool.tile([128, 512], F16, tag="h2")
                eng2 = nc.gpsimd if (t % 2 == 0) else nc.vector
                eng2.tensor_scalar(h2, t2, C2, C2, ALU.add, ALU.subtract)

                nc.tensor.matmul(ps3, w3_sb[:, 128 * tau:128 * (tau + 1)], h2,
                                 start=(tau == 0), stop=(tau == 15))

            t3 = tpool.tile([128, 512], F32, tag="t3")
            nc.scalar.activation(t3, ps3, AF.Identity, bias=bc3_sb, scale=1.0)
            y = tpool.tile([128, 512], F32, tag="y")
            nc.vector.tensor_scalar(y, t3, C2, C2, ALU.add, ALU.subtract)
            oa = opool.tile([128, 512], F32, tag="oa")
            nc.scalar.activation(oa, y, AF.Copy, bias=0.0, scale=1.0 / 256.0)
            e = tpool.tile([128, 512], F32, tag="e")
            nc.scalar.activation(e, y, AF.Exp, bias=bm4_sb, scale=1.0 / 256.0)
            ob = opool.tile([128, 512], F32, tag="ob")
            nc.vector.tensor_scalar(ob, e, float(np.exp(5.0)),
                                    float(np.exp(-4.6)), ALU.min, ALU.max)
            nc.sync.dma_start(out=outa[pack], in_=oa)
            nc.sync.dma_start(out=outb[pack], in_=ob)

    nc.compile()
    return nc


def _get_graph():
    if "nc" not in _compiled:
        _compiled["nc"] = _build_graph()
    return _compiled["nc"]


def _prep_weights(w0, b0, w1, b1, w_out, b_out):
    eye = np.eye(C, dtype=np.float32)
    W0s = (w0.T.astype(np.float32) + 256.0 * eye)
    W1s_small = ((w1.T.astype(np.float32) + 256.0 * eye) / 256.0).astype(np.float16)
    W3_small = (w_out.T.astype(np.float32) / 256.0).astype(np.float16)  # [32, 2]

    w1s = np.zeros((128, 128), np.float32)
    w2s = np.zeros((128, 128), np.float16)
    for b in range(4):
        w1s[32 * b:32 * b + 32, 32 * b:32 * b + 32] = W0s
        w2s[32 * b:32 * b + 32, 32 * b:32 * b + 32] = W1s_small

    # w3s[32 b + c, 128 tau + (8 tau'? no) ...]: stationary for pack matmuls.
    # For inner-loop index tau: out partition m = 8 tau + 4 o + b.
    w3pack = np.zeros((16, 128, 128), np.float16)
    for tau in range(16):
        for b in range(4):
            for o in range(2):
                w3pack[tau, 32 * b:32 * b + 32, 8 * tau + 4 * o + b] = W3_small[:, o]
    # device loads w3s as one [128, 2048] tile, slice [:, 128 tau:128(tau+1)]
    w3s = np.ascontiguousarray(w3pack.transpose(1, 0, 2).reshape(128, 2048))

    bc1 = np.zeros((128, 1), np.float32)
    bc2 = np.zeros((128, 1), np.float32)
    bc3 = np.zeros((128, 1), np.float32)
    for b in range(4):
        bc1[32 * b:32 * b + 32, 0] = b0.astype(np.float32) / 256.0 + CTIE
        bc2[32 * b:32 * b + 32, 0] = b1.astype(np.float32) / 256.0 + CTIE
    for tau in range(16):
        for o in range(2):
            for b in range(4):
                bc3[8 * tau + 4 * o + b, 0] = float(b_out[o]) / 256.0 + CTIE
    return w1s, w2s, w3s, bc1, bc2, bc3


def kernel(x, w0, b0, w1, b1, w_out, b_out):
    x = np.ascontiguousarray(np.asarray(x, np.float32))
    w1s, w2s, w3s, bc1, bc2, bc3 = _prep_weights(
        np.asarray(w0), np.asarray(b0), np.asarray(w1), np.asarray(b1),
        np.asarray(w_out), np.asarray(b_out))

    nc = _get_graph()

    in_maps = []
    for i in range(NCORES):
        xs = x[i * S:(i + 1) * S]                       # [S, 32]
        # xd[t, 32 b + c, f'] = xs[2048 t + 512 b + f', c]
        xd = np.ascontiguousarray(
            xs.reshape(NT, 4, 512, C).transpose(0, 1, 3, 2).reshape(NT, 128, 512))
        in_maps.append({"xd": xd, "w1s": w1s, "w2s": w2s, "w3s": w3s,
                        "bc1": bc1, "bc2": bc2, "bc3": bc3})

    res = run_bass_kernel_spmd(nc, in_maps, list(range(NCORES))).results

    mu = np.empty(B, np.float32)
    ls = np.empty(B, np.float32)
    sc = np.empty(B, np.float32)
    for i in range(NCORES):
        # outa[pack, 8 tau + 4 o + b, f'] = raw(row = 2048(16 pack+tau)+512 b+f', o)
        a = np.asarray(res[i]["outa"]).reshape(NPACK, 16, 2, 4, 512)
        bb = np.asarray(res[i]["outb"]).reshape(NPACK, 16, 2, 4, 512)
        sl = slice(i * S, (i + 1) * S)
        mu[sl] = a[:, :, 0].reshape(S)
        ls[sl] = a[:, :, 1].reshape(S)
        sc[sl] = bb[:, :, 1].reshape(S)
    return mu, sc, ls


if __name__ == "__main__":
    rng = np.random.default_rng(0)
    x = rng.standard_normal((B, C)).astype(np.float32)
    w0 = np.round(rng.standard_normal((C, C)) * 13).astype(np.float32)
    b0 = np.round(rng.standard_normal(C) * 3000).astype(np.float32)
    w1 = np.round(rng.standard_normal((C, C)) * 13).astype(np.float32)
    b1 = np.round(rng.standard_normal(C) * 3000).astype(np.float32)
    w_out = np.round(rng.standard_normal((2, C)) * 13).astype(np.float32)
    b_out = np.round(rng.standard_normal(2) * 3000).astype(np.float32)
    out = kernel(x, w0, b0, w1, b1, w_out, b_out)
    print([o.shape for o in out], [float(np.abs(o).mean()) for o in out])


# revision 4
# speedup vs baseline: 23007.3332x; 23007.3332x over previous
"""Trainium2 Bass kernel for nn_ArmInt_19911468384433 (dense_mlp, 8 cores).

Data-parallel: x [2097152, 32] sharded by rows across 8 NeuronCores; tiny
32x32 weights folded/replicated. All math (3 integer-MLP layers with
emulated fixed-point rounding, exp/clip head) runs on device. Host does
layout permutation (shard + transpose-pack of input, fp16 hi/lo split,
inverse reshape of output) and weight folding only.

Device algorithm per core (S = 262144 rows = 128 tiles of 2048 rows):
  x pre-packed on host as xd*[t, 32b+c, f'] = x[2048 t + 512 b + f', c],
  split x = xhi + xlo (both fp16; 22-bit combined mantissa, exact enough:
  boundary-flip rate below the fp32 accumulation-order noise floor).
  Per tile:
    mm1 : ps1 = W1s.T @ xhi + W1s.T @ xlo   (fp16, PSUM f32 accumulate)
    ep1 : t = Relu(ps1 + bc1) (ACT, bias AP)  then h1 = rne(t) via DVE
          fused ts (+2^23, -2^23) -> fp16   [or the all-DVE form below]
    mm2 : ps2 = W2s.T @ h1  (fp16);  ep2 likewise -> h2
    mm3 : ps3pack += W3s_tau.T @ h2  (16 tiles accumulate into one bank)
  Every 4th layer-slot uses the all-DVE form to balance engines:
    u = ts(ps + bc, + 2^23) ; h = ts(u - 2^23, max 0) -> fp16
  Per 16-tile pack (DVE except Exp):
    t3 = ps3pack + bc3 ; y = rne(t3) ; outA = y/256
    e = Exp(y/256 - 4) (ACT) ; outB = clip(e, e^-4.6, e^5)

rne(v + 2^-9) == trunc-round-half-away-from-zero on the 1/256-granular
values here; verified vs reference at rel err ~5e-3 (fp32 order noise).
"""
import sys

sys.path.insert(0, "/opt/trn_rl_repo")

from contextlib import ExitStack

import numpy as np

import concourse.bacc as bacc
import concourse.bass as bass
import concourse.tile as tile
from concourse import mybir
from concourse.bass_utils import run_bass_kernel_spmd

F32 = mybir.dt.float32
F16 = mybir.dt.float16
AF = mybir.ActivationFunctionType
ALU = mybir.AluOpType

B = 2097152
C = 32
NCORES = 8
S = B // NCORES            # 262144 rows per core
NT = S // 2048             # 128 tiles per core
NPACK = NT // 16           # 8 packs per core
C2 = float(2.0 ** 23)
CTIE = float(2.0 ** -9)

_compiled = {}


def _layer_ep(nc, pools, ps, bc_sb, bcm_sb, out_dt, dve_form, tagp):
    """PSUM -> relu(rne(v + bc)) -> SBUF tile (out_dt)."""
    tpool, hpool = pools
    if not dve_form:
        t = tpool.tile([128, 512], F32, tag=tagp + "t", name="t")
        nc.scalar.activation(t, ps, AF.Relu, bias=bc_sb, scale=1.0)
        h = hpool.tile([128, 512], out_dt, tag=tagp + "h", name="h")
        nc.vector.tensor_scalar(h, t, C2, C2, ALU.add, ALU.subtract)
    else:
        u = tpool.tile([128, 512], F32, tag=tagp + "t", name="u")
        nc.vector.tensor_scalar(u, ps, bc_sb, C2, ALU.add, ALU.add)
        h = hpool.tile([128, 512], out_dt, tag=tagp + "h", name="h")
        nc.vector.tensor_scalar(h, u, C2, 0.0, ALU.subtract, ALU.max)
    return h


def _build_graph():
    nc = bacc.Bacc("TRN2", target_bir_lowering=False, debug=False)
    xhi = nc.declare_dram_parameter("xhi", [NT, 128, 512], F16, isOutput=False)
    xlo = nc.declare_dram_parameter("xlo", [NT, 128, 512], F16, isOutput=False)
    w1s = nc.declare_dram_parameter("w1s", [128, 128], F16, isOutput=False)
    w2s = nc.declare_dram_parameter("w2s", [128, 128], F16, isOutput=False)
    w3s = nc.declare_dram_parameter("w3s", [128, 2048], F16, isOutput=False)
    bc1 = nc.declare_dram_parameter("bc1", [128, 1], F32, isOutput=False)
    bc2 = nc.declare_dram_parameter("bc2", [128, 1], F32, isOutput=False)
    bc3 = nc.declare_dram_parameter("bc3", [128, 1], F32, isOutput=False)
    outa = nc.declare_dram_parameter("outa", [NPACK, 128, 512], F32, isOutput=True)
    outb = nc.declare_dram_parameter("outb", [NPACK, 128, 512], F32, isOutput=True)

    with ExitStack() as ctx:
        tc = ctx.enter_context(tile.TileContext(nc))
        consts = ctx.enter_context(tc.tile_pool(name="consts", bufs=1))
        xpool = ctx.enter_context(tc.tile_pool(name="xpool", bufs=3))
        tpool = ctx.enter_context(tc.tile_pool(name="tpool", bufs=3))
        hpool = ctx.enter_context(tc.tile_pool(name="hpool", bufs=3))
        opool = ctx.enter_context(tc.tile_pool(name="opool", bufs=2))
        ps1p = ctx.enter_context(tc.tile_pool(name="ps1p", bufs=2, space="PSUM"))
        ps2p = ctx.enter_context(tc.tile_pool(name="ps2p", bufs=2, space="PSUM"))
        ps3p = ctx.enter_context(tc.tile_pool(name="ps3p", bufs=2, space="PSUM"))

        w1_sb = consts.tile([128, 128], F16, tag="w1", name="w1_sb")
        nc.gpsimd.dma_start(out=w1_sb, in_=w1s[:])
        w2_sb = consts.tile([128, 128], F16, tag="w2", name="w2_sb")
        nc.gpsimd.dma_start(out=w2_sb, in_=w2s[:])
        w3_sb = consts.tile([128, 2048], F16, tag="w3", name="w3_sb")
        nc.gpsimd.dma_start(out=w3_sb, in_=w3s[:])
        bc1_sb = consts.tile([128, 1], F32, tag="bc1", name="bc1_sb")
        nc.gpsimd.dma_start(out=bc1_sb, in_=bc1[:])
        bc2_sb = consts.tile([128, 1], F32, tag="bc2", name="bc2_sb")
        nc.gpsimd.dma_start(out=bc2_sb, in_=bc2[:])
        bc3_sb = consts.tile([128, 1], F32, tag="bc3", name="bc3_sb")
        nc.gpsimd.dma_start(out=bc3_sb, in_=bc3[:])
        bm4_sb = consts.tile([128, 1], F32, tag="bm4", name="bm4_sb")
        nc.vector.memset(bm4_sb, -4.0)

        pools = (tpool, hpool)
        for pack in range(NPACK):
            ps3 = ps3p.tile([128, 512], F32, tag="ps3", name="ps3")
            for tau in range(16):
                t = pack * 16 + tau
                xh = xpool.tile([128, 512], F16, tag="xh", name="xh")
                nc.sync.dma_start(out=xh, in_=xhi[t])
                xl = xpool.tile([128, 512], F16, tag="xl", name="xl")
                nc.sync.dma_start(out=xl, in_=xlo[t])

                ps1 = ps1p.tile([128, 512], F32, tag="ps1", name="ps1")
                nc.tensor.matmul(ps1, w1_sb, xh, start=True, stop=False)
                nc.tensor.matmul(ps1, w1_sb, xl, start=False, stop=True)
                # every 8th layer-slot (2 per tile) takes the all-DVE form
                slot = 2 * t
                h1 = _layer_ep(nc, pools, ps1, bc1_sb, bm4_sb, F16,
                               dve_form=(slot % 8 == 7), tagp="l1")

                ps2 = ps2p.tile([128, 512], F32, tag="ps2", name="ps2")
                nc.tensor.matmul(ps2, w2_sb, h1, start=True, stop=True)
                h2 = _layer_ep(nc, pools, ps2, bc2_sb, bm4_sb, F16,
                               dve_form=((slot + 1) % 8 == 7), tagp="l2")

                nc.tensor.matmul(ps3, w3_sb[:, 128 * tau:128 * (tau + 1)], h2,
                                 start=(tau == 0), stop=(tau == 15))

            t3 = tpool.tile([128, 512], F32, tag="t3", name="t3")
            nc.vector.tensor_scalar(t3, ps3, bc3_sb, C2, ALU.add, ALU.add)
            y = tpool.tile([128, 512], F32, tag="y", name="y")
            nc.vector.tensor_scalar(y, t3, C2, None, ALU.subtract)
            oa = opool.tile([128, 512], F32, tag="oa", name="oa")
            nc.vector.tensor_scalar(oa, y, 1.0 / 256.0, None, ALU.mult)
            e = tpool.tile([128, 512], F32, tag="e", name="e")
            nc.scalar.activation(e, y, AF.Exp, bias=bm4_sb, scale=1.0 / 256.0)
            ob = opool.tile([128, 512], F32, tag="ob", name="ob")
            nc.vector.tensor_scalar(ob, e, float(np.exp(5.0)),
                                    float(np.exp(-4.6)), ALU.min, ALU.max)
            nc.sync.dma_start(out=outa[pack], in_=oa)
            nc.sync.dma_start(out=outb[pack], in_=ob)

    nc.compile()
    return nc


def _get_graph():
    if "nc" not in _compiled:
        _compiled["nc"] = _build_graph()
    return _compiled["nc"]


def _prep_weights(w0, b0, w1, b1, w_out, b_out):
    eye = np.eye(C, dtype=np.float32)
    W0s = (w0.T.astype(np.float32) + 256.0 * eye).astype(np.float16)
    W1s_small = ((w1.T.astype(np.float32) + 256.0 * eye) / 256.0).astype(np.float16)
    W3_small = (w_out.T.astype(np.float32) / 256.0).astype(np.float16)  # [32, 2]

    w1s = np.zeros((128, 128), np.float16)
    w2s = np.zeros((128, 128), np.float16)
    for b in range(4):
        w1s[32 * b:32 * b + 32, 32 * b:32 * b + 32] = W0s
        w2s[32 * b:32 * b + 32, 32 * b:32 * b + 32] = W1s_small

    # mm3 stationary for inner-loop index tau: out partition m = 8 tau + 4 o + b
    w3pack = np.zeros((16, 128, 128), np.float16)
    for tau in range(16):
        for b in range(4):
            for o in range(2):
                w3pack[tau, 32 * b:32 * b + 32, 8 * tau + 4 * o + b] = W3_small[:, o]
    w3s = np.ascontiguousarray(w3pack.transpose(1, 0, 2).reshape(128, 2048))

    bc1 = np.zeros((128, 1), np.float32)
    bc2 = np.zeros((128, 1), np.float32)
    bc3 = np.zeros((128, 1), np.float32)
    for b in range(4):
        bc1[32 * b:32 * b + 32, 0] = b0.astype(np.float32) / 256.0 + CTIE
        bc2[32 * b:32 * b + 32, 0] = b1.astype(np.float32) / 256.0 + CTIE
    for tau in range(16):
        for o in range(2):
            for b in range(4):
                bc3[8 * tau + 4 * o + b, 0] = float(b_out[o]) / 256.0 + CTIE
    return w1s, w2s, w3s, bc1, bc2, bc3


def _prep_x_core(xs):
    """[S, 32] f32 -> (xhi, xlo) [NT, 128, 512] fp16 in device layout."""
    xd = np.ascontiguousarray(
        xs.reshape(NT, 4, 512, C).transpose(0, 1, 3, 2).reshape(NT, 128, 512))
    xh = xd.astype(np.float16)
    xl = (xd - xh.astype(np.float32)).astype(np.float16)
    return xh, xl


def kernel(x, w0, b0, w1, b1, w_out, b_out):
    x = np.ascontiguousarray(np.asarray(x, np.float32))
    w1s, w2s, w3s, bc1, bc2, bc3 = _prep_weights(
        np.asarray(w0), np.asarray(b0), np.asarray(w1), np.asarray(b1),
        np.asarray(w_out), np.asarray(b_out))

    nc = _get_graph()

    in_maps = []
    for i in range(NCORES):
        xh, xl = _prep_x_core(x[i * S:(i + 1) * S])
        in_maps.append({"xhi": xh, "xlo": xl, "w1s": w1s, "w2s": w2s,
                        "w3s": w3s, "bc1": bc1, "bc2": bc2, "bc3": bc3})

    res = run_bass_kernel_spmd(nc, in_maps, list(range(NCORES))).results

    mu = np.empty(B, np.float32)
    ls = np.empty(B, np.float32)
    sc = np.empty(B, np.float32)
    for i in range(NCORES):
        # outa[pack, 8 tau + 4 o + b, f'] = raw(row = 2048(16 pack+tau)+512 b+f', o)
        a = np.asarray(res[i]["outa"]).reshape(NPACK, 16, 2, 4, 512)
        bb = np.asarray(res[i]["outb"]).reshape(NPACK, 16, 2, 4, 512)
        sl = slice(i * S, (i + 1) * S)
        mu[sl] = a[:, :, 0].reshape(S)
        ls[sl] = a[:, :, 1].reshape(S)
        sc[sl] = bb[:, :, 1].reshape(S)
    return mu, sc, ls


if __name__ == "__main__":
    rng = np.random.default_rng(0)
    x = rng.standard_normal((B, C)).astype(np.float32)
    w0 = np.round(rng.standard_normal((C, C)) * 13).astype(np.float32)
    b0 = np.round(rng.standard_normal(C) * 3000).astype(np.float32)
    w1 = np.round(rng.standard_normal((C, C)) * 13).astype(np.float32)
    b1 = np.round(rng.standard_normal(C) * 3000).astype(np.float32)
    w_out = np.round(rng.standard_normal((2, C)) * 13).astype(np.float32)
    b_out = np.round(rng.standard_normal(2) * 3000).astype(np.float32)
    out = kernel(x, w0, b0, w1, b1, w_out, b_out)
    print([o.shape for o in out], [float(np.abs(o).mean()) for o in out])


# revision 8
# speedup vs baseline: 31257.9536x; 1.3586x over previous
"""Trainium2 Bass kernel for nn_ArmInt_19911468384433 (dense_mlp, 8 cores).

Data-parallel: x [2097152, 32] sharded by rows across 8 NeuronCores; tiny
32x32 weights folded/replicated. All math (3 integer-MLP layers with
emulated fixed-point rounding, exp/clip head) runs on device. Host does
layout permutation (shard + transpose-pack of input, fp16 hi/lo split,
inverse reshape of output) and weight folding only.

Device algorithm per core (S = 262144 rows = 128 tiles of 2048 rows):
  x pre-packed on host as xd*[t, 32b+c, f'] = x[2048 t + 512 b + f', c],
  split x = xhi + xlo (both fp16; 22-bit combined mantissa, exact enough:
  boundary-flip rate below the fp32 accumulation-order noise floor).
  Per tile:
    mm1 : ps1 = W1s.T @ xhi + W1s.T @ xlo   (fp16, PSUM f32 accumulate)
    ep1 : t = Relu(ps1 + bc1) (ACT, bias AP)  then h1 = rne(t) via DVE
          fused ts (+2^23, -2^23) -> fp16   [or the all-DVE form below]
    mm2 : ps2 = W2s.T @ h1  (fp16);  ep2 likewise -> h2
    mm3 : ps3pack += W3s_tau.T @ h2  (16 tiles accumulate into one bank)
  Every 4th layer-slot uses the all-DVE form to balance engines:
    u = ts(ps + bc, + 2^23) ; h = ts(u - 2^23, max 0) -> fp16
  Per 16-tile pack (DVE except Exp):
    t3 = ps3pack + bc3 ; y = rne(t3) ; outA = y/256
    e = Exp(y/256 - 4) (ACT) ; outB = clip(e, e^-4.6, e^5)

rne(v + 2^-9) == trunc-round-half-away-from-zero on the 1/256-granular
values here; verified vs reference at rel err ~5e-3 (fp32 order noise).
"""
import sys

sys.path.insert(0, "/opt/trn_rl_repo")

from contextlib import ExitStack

import numpy as np

import concourse.bacc as bacc
import concourse.bass as bass
import concourse.tile as tile
from concourse import mybir
from concourse.bass_utils import run_bass_kernel_spmd

F32 = mybir.dt.float32
F16 = mybir.dt.float16
AF = mybir.ActivationFunctionType
ALU = mybir.AluOpType

B = 2097152
C = 32
NCORES = 8
S = B // NCORES            # 262144 rows per core
NT = S // 2048             # 128 tiles per core
NPACK = NT // 16           # 8 packs per core
C2 = float(2.0 ** 23)
CTIE = float(2.0 ** -9)

_compiled = {}


def _layer_ep(nc, pools, ps, bc_sb, bcm_sb, out_dt, dve_form, tagp):
    """PSUM -> relu(rne(v + bc)) -> SBUF tile (out_dt)."""
    tpool, hpool = pools
    if not dve_form:
        t = tpool.tile([128, 512], F32, tag=tagp + "t", name="t")
        nc.scalar.activation(t, ps, AF.Relu, bias=bc_sb, scale=1.0)
        h = hpool.tile([128, 512], out_dt, tag=tagp + "h", name="h")
        nc.vector.tensor_scalar(h, t, C2, C2, ALU.add, ALU.subtract)
    else:
        u = tpool.tile([128, 512], F32, tag=tagp + "t", name="u")
        nc.vector.tensor_scalar(u, ps, bc_sb, C2, ALU.add, ALU.add)
        h = hpool.tile([128, 512], out_dt, tag=tagp + "h", name="h")
        nc.vector.tensor_scalar(h, u, C2, 0.0, ALU.subtract, ALU.max)
    return h


def _build_graph():
    nc = bacc.Bacc("TRN2", target_bir_lowering=False, debug=False)
    # hi/lo fp16 halves for two consecutive tiles packed in one 512KB DMA:
    # [:, 0:512]=hi(2j) [:, 512:1024]=lo(2j) [:, 1024:1536]=hi(2j+1) ...
    xpk = nc.declare_dram_parameter("xpk", [NT // 2, 128, 2048], F16, isOutput=False)
    w1s = nc.declare_dram_parameter("w1s", [128, 128], F16, isOutput=False)
    w2s = nc.declare_dram_parameter("w2s", [128, 128], F16, isOutput=False)
    w3s = nc.declare_dram_parameter("w3s", [128, 2048], F16, isOutput=False)
    bc1 = nc.declare_dram_parameter("bc1", [128, 1], F32, isOutput=False)
    bc2 = nc.declare_dram_parameter("bc2", [128, 1], F32, isOutput=False)
    bc3 = nc.declare_dram_parameter("bc3", [128, 1], F32, isOutput=False)
    outa = nc.declare_dram_parameter("outa", [NPACK, 128, 512], F32, isOutput=True)
    outb = nc.declare_dram_parameter("outb", [NPACK, 128, 512], F32, isOutput=True)

    with ExitStack() as ctx:
        tc = ctx.enter_context(tile.TileContext(nc))
        consts = ctx.enter_context(tc.tile_pool(name="consts", bufs=1))
        xpool = ctx.enter_context(tc.tile_pool(name="xpool", bufs=3))
        tpool = ctx.enter_context(tc.tile_pool(name="tpool", bufs=3))
        hpool = ctx.enter_context(tc.tile_pool(name="hpool", bufs=3))
        opool = ctx.enter_context(tc.tile_pool(name="opool", bufs=2))
        ps1p = ctx.enter_context(tc.tile_pool(name="ps1p", bufs=2, space="PSUM"))
        ps2p = ctx.enter_context(tc.tile_pool(name="ps2p", bufs=2, space="PSUM"))
        ps3p = ctx.enter_context(tc.tile_pool(name="ps3p", bufs=2, space="PSUM"))

        w1_sb = consts.tile([128, 128], F16, tag="w1", name="w1_sb")
        nc.gpsimd.dma_start(out=w1_sb, in_=w1s[:])
        w2_sb = consts.tile([128, 128], F16, tag="w2", name="w2_sb")
        nc.gpsimd.dma_start(out=w2_sb, in_=w2s[:])
        w3_sb = consts.tile([128, 2048], F16, tag="w3", name="w3_sb")
        nc.gpsimd.dma_start(out=w3_sb, in_=w3s[:])
        bc1_sb = consts.tile([128, 1], F32, tag="bc1", name="bc1_sb")
        nc.gpsimd.dma_start(out=bc1_sb, in_=bc1[:])
        bc2_sb = consts.tile([128, 1], F32, tag="bc2", name="bc2_sb")
        nc.gpsimd.dma_start(out=bc2_sb, in_=bc2[:])
        bc3_sb = consts.tile([128, 1], F32, tag="bc3", name="bc3_sb")
        nc.gpsimd.dma_start(out=bc3_sb, in_=bc3[:])
        bm4_sb = consts.tile([128, 1], F32, tag="bm4", name="bm4_sb")
        nc.vector.memset(bm4_sb, -4.0)

        pools = (tpool, hpool)
        for pack in range(NPACK):
            ps3 = ps3p.tile([128, 512], F32, tag="ps3", name="ps3")
            for tau in range(16):
                t = pack * 16 + tau
                if tau % 2 == 0:
                    xt2 = xpool.tile([128, 2048], F16, tag="xt2", name="xt2")
                    nc.sync.dma_start(out=xt2, in_=xpk[t // 2])
                off = 1024 * (tau % 2)
                xh = xt2[:, off:off + 512]
                xl = xt2[:, off + 512:off + 1024]

                ps1 = ps1p.tile([128, 512], F32, tag="ps1", name="ps1")
                nc.tensor.matmul(ps1, w1_sb, xh, start=True, stop=False)
                nc.tensor.matmul(ps1, w1_sb, xl, start=False, stop=True)
                # every 8th layer-slot (2 per tile) takes the all-DVE form
                slot = 2 * t
                h1 = _layer_ep(nc, pools, ps1, bc1_sb, bm4_sb, F16,
                               dve_form=(slot % 8 == 7), tagp="l1")

                ps2 = ps2p.tile([128, 512], F32, tag="ps2", name="ps2")
                nc.tensor.matmul(ps2, w2_sb, h1, start=True, stop=True)
                h2 = _layer_ep(nc, pools, ps2, bc2_sb, bm4_sb, F16,
                               dve_form=((slot + 1) % 8 == 7), tagp="l2")

                nc.tensor.matmul(ps3, w3_sb[:, 128 * tau:128 * (tau + 1)], h2,
                                 start=(tau == 0), stop=(tau == 15))

            t3 = tpool.tile([128, 512], F32, tag="t3", name="t3")
            nc.vector.tensor_scalar(t3, ps3, bc3_sb, C2, ALU.add, ALU.add)
            y = tpool.tile([128, 512], F32, tag="y", name="y")
            nc.vector.tensor_scalar(y, t3, C2, None, ALU.subtract)
            oa = opool.tile([128, 512], F32, tag="oa", name="oa")
            nc.vector.tensor_scalar(oa, y, 1.0 / 256.0, None, ALU.mult)
            e = tpool.tile([128, 512], F32, tag="e", name="e")
            nc.scalar.activation(e, y, AF.Exp, bias=bm4_sb, scale=1.0 / 256.0)
            ob = opool.tile([128, 512], F32, tag="ob", name="ob")
            nc.vector.tensor_scalar(ob, e, float(np.exp(5.0)),
                                    float(np.exp(-4.6)), ALU.min, ALU.max)
            nc.sync.dma_start(out=outa[pack], in_=oa)
            nc.sync.dma_start(out=outb[pack], in_=ob)

    nc.compile()
    return nc


def _get_graph():
    if "nc" not in _compiled:
        _compiled["nc"] = _build_graph()
    return _compiled["nc"]


def _prep_weights(w0, b0, w1, b1, w_out, b_out):
    eye = np.eye(C, dtype=np.float32)
    W0s = (w0.T.astype(np.float32) + 256.0 * eye).astype(np.float16)
    W1s_small = ((w1.T.astype(np.float32) + 256.0 * eye) / 256.0).astype(np.float16)
    W3_small = (w_out.T.astype(np.float32) / 256.0).astype(np.float16)  # [32, 2]

    w1s = np.zeros((128, 128), np.float16)
    w2s = np.zeros((128, 128), np.float16)
    for b in range(4):
        w1s[32 * b:32 * b + 32, 32 * b:32 * b + 32] = W0s
        w2s[32 * b:32 * b + 32, 32 * b:32 * b + 32] = W1s_small

    # mm3 stationary for inner-loop index tau: out partition m = 8 tau + 4 o + b
    w3pack = np.zeros((16, 128, 128), np.float16)
    for tau in range(16):
        for b in range(4):
            for o in range(2):
                w3pack[tau, 32 * b:32 * b + 32, 8 * tau + 4 * o + b] = W3_small[:, o]
    w3s = np.ascontiguousarray(w3pack.transpose(1, 0, 2).reshape(128, 2048))

    bc1 = np.zeros((128, 1), np.float32)
    bc2 = np.zeros((128, 1), np.float32)
    bc3 = np.zeros((128, 1), np.float32)
    for b in range(4):
        bc1[32 * b:32 * b + 32, 0] = b0.astype(np.float32) / 256.0 + CTIE
        bc2[32 * b:32 * b + 32, 0] = b1.astype(np.float32) / 256.0 + CTIE
    for tau in range(16):
        for o in range(2):
            for b in range(4):
                bc3[8 * tau + 4 * o + b, 0] = float(b_out[o]) / 256.0 + CTIE
    return w1s, w2s, w3s, bc1, bc2, bc3


def _prep_x_core(xs):
    """[S, 32] f32 -> xpk [NT//2, 128, 2048] fp16 (hi/lo, 2 tiles per row)."""
    xd = xs.reshape(NT, 4, 512, C).transpose(0, 1, 3, 2).reshape(NT, 128, 512)
    xh = xd.astype(np.float16)
    xl = (xd - xh.astype(np.float32)).astype(np.float16)
    xpk = np.empty((NT // 2, 128, 2048), np.float16)
    xpk[:, :, 0:512] = xh[0::2]
    xpk[:, :, 512:1024] = xl[0::2]
    xpk[:, :, 1024:1536] = xh[1::2]
    xpk[:, :, 1536:2048] = xl[1::2]
    return xpk


def kernel(x, w0, b0, w1, b1, w_out, b_out):
    x = np.ascontiguousarray(np.asarray(x, np.float32))
    w1s, w2s, w3s, bc1, bc2, bc3 = _prep_weights(
        np.asarray(w0), np.asarray(b0), np.asarray(w1), np.asarray(b1),
        np.asarray(w_out), np.asarray(b_out))

    nc = _get_graph()

    in_maps = []
    for i in range(NCORES):
        xpk = _prep_x_core(x[i * S:(i + 1) * S])
        in_maps.append({"xpk": xpk, "w1s": w1s, "w2s": w2s,
                        "w3s": w3s, "bc1": bc1, "bc2": bc2, "bc3": bc3})

    res = run_bass_kernel_spmd(nc, in_maps, list(range(NCORES))).results

    mu = np.empty(B, np.float32)
    ls = np.empty(B, np.float32)
    sc = np.empty(B, np.float32)
    for i in range(NCORES):
        # outa[pack, 8 tau + 4 o + b, f'] = raw(row = 2048(16 pack+tau)+512 b+f', o)
        a = np.asarray(res[i]["outa"]).reshape(NPACK, 16, 2, 4, 512)
        bb = np.asarray(res[i]["outb"]).reshape(NPACK, 16, 2, 4, 512)
        sl = slice(i * S, (i + 1) * S)
        mu[sl] = a[:, :, 0].reshape(S)
        ls[sl] = a[:, :, 1].reshape(S)
        sc[sl] = bb[:, :, 1].reshape(S)
    return mu, sc, ls


if __name__ == "__main__":
    rng = np.random.default_rng(0)
    x = rng.standard_normal((B, C)).astype(np.float32)
    w0 = np.round(rng.standard_normal((C, C)) * 13).astype(np.float32)
    b0 = np.round(rng.standard_normal(C) * 3000).astype(np.float32)
    w1 = np.round(rng.standard_normal((C, C)) * 13).astype(np.float32)
    b1 = np.round(rng.standard_normal(C) * 3000).astype(np.float32)
    w_out = np.round(rng.standard_normal((2, C)) * 13).astype(np.float32)
    b_out = np.round(rng.standard_normal(2) * 3000).astype(np.float32)
    out = kernel(x, w0, b0, w1, b1, w_out, b_out)
    print([o.shape for o in out], [float(np.abs(o).mean()) for o in out])


# revision 9
# speedup vs baseline: 31777.8418x; 1.0166x over previous
"""Trainium2 Bass kernel for nn_ArmInt_19911468384433 (dense_mlp, 8 cores).

Data-parallel: x [2097152, 32] sharded by rows across 8 NeuronCores; tiny
32x32 weights folded/replicated. All math (3 integer-MLP layers with
emulated fixed-point rounding, exp/clip head) runs on device. Host does
layout permutation (shard + transpose-pack of input, fp16 hi/lo split,
inverse reshape of output) and weight folding only.

Device algorithm per core (S = 262144 rows = 128 tiles of 2048 rows):
  x pre-packed on host as xd*[t, 32b+c, f'] = x[2048 t + 512 b + f', c],
  split x = xhi + xlo (both fp16; 22-bit combined mantissa, exact enough:
  boundary-flip rate below the fp32 accumulation-order noise floor).
  Per tile:
    mm1 : ps1 = W1s.T @ xhi + W1s.T @ xlo   (fp16, PSUM f32 accumulate)
    ep1 : t = Relu(ps1 + bc1) (ACT, bias AP)  then h1 = rne(t) via DVE
          fused ts (+2^23, -2^23) -> fp16   [or the all-DVE form below]
    mm2 : ps2 = W2s.T @ h1  (fp16);  ep2 likewise -> h2
    mm3 : ps3pack += W3s_tau.T @ h2  (16 tiles accumulate into one bank)
  Every 4th layer-slot uses the all-DVE form to balance engines:
    u = ts(ps + bc, + 2^23) ; h = ts(u - 2^23, max 0) -> fp16
  Per 16-tile pack (DVE except Exp):
    t3 = ps3pack + bc3 ; y = rne(t3) ; outA = y/256
    e = Exp(y/256 - 4) (ACT) ; outB = clip(e, e^-4.6, e^5)

rne(v + 2^-9) == trunc-round-half-away-from-zero on the 1/256-granular
values here; verified vs reference at rel err ~5e-3 (fp32 order noise).
"""
import sys

sys.path.insert(0, "/opt/trn_rl_repo")

from contextlib import ExitStack

import numpy as np

import concourse.bacc as bacc
import concourse.bass as bass
import concourse.tile as tile
from concourse import mybir
from concourse.bass_utils import run_bass_kernel_spmd

F32 = mybir.dt.float32
F16 = mybir.dt.float16
AF = mybir.ActivationFunctionType
ALU = mybir.AluOpType

B = 2097152
C = 32
NCORES = 8
S = B // NCORES            # 262144 rows per core
NT = S // 2048             # 128 tiles per core
NPACK = NT // 16           # 8 packs per core
C2 = float(2.0 ** 23)
CTIE = float(2.0 ** -9)

_compiled = {}


def _layer_ep(nc, pools, ps, bc_sb, bcm_sb, out_dt, dve_form, tagp):
    """PSUM -> relu(rne(v + bc)) -> SBUF tile (out_dt)."""
    tpool, hpool = pools
    if not dve_form:
        t = tpool.tile([128, 512], F32, tag=tagp + "t", name="t")
        nc.scalar.activation(t, ps, AF.Relu, bias=bc_sb, scale=1.0)
        h = hpool.tile([128, 512], out_dt, tag=tagp + "h", name="h")
        nc.vector.tensor_scalar(h, t, C2, C2, ALU.add, ALU.subtract)
    else:
        u = tpool.tile([128, 512], F32, tag=tagp + "t", name="u")
        nc.vector.tensor_scalar(u, ps, bc_sb, C2, ALU.add, ALU.add)
        h = hpool.tile([128, 512], out_dt, tag=tagp + "h", name="h")
        nc.vector.tensor_scalar(h, u, C2, 0.0, ALU.subtract, ALU.max)
    return h


def _build_graph():
    nc = bacc.Bacc("TRN2", target_bir_lowering=False, debug=False)
    # hi/lo fp16 halves for two consecutive tiles packed in one 512KB DMA:
    # [:, 0:512]=hi(2j) [:, 512:1024]=lo(2j) [:, 1024:1536]=hi(2j+1) ...
    xpk = nc.declare_dram_parameter("xpk", [NT // 2, 128, 2048], F16, isOutput=False)
    w1s = nc.declare_dram_parameter("w1s", [128, 128], F16, isOutput=False)
    w2s = nc.declare_dram_parameter("w2s", [128, 128], F16, isOutput=False)
    w3s = nc.declare_dram_parameter("w3s", [128, 2048], F16, isOutput=False)
    bc1 = nc.declare_dram_parameter("bc1", [128, 1], F32, isOutput=False)
    bc2 = nc.declare_dram_parameter("bc2", [128, 1], F32, isOutput=False)
    bc3 = nc.declare_dram_parameter("bc3", [128, 1], F32, isOutput=False)
    outa = nc.declare_dram_parameter("outa", [NPACK, 128, 512], F32, isOutput=True)
    outb = nc.declare_dram_parameter("outb", [NPACK, 128, 512], F32, isOutput=True)

    with ExitStack() as ctx:
        tc = ctx.enter_context(tile.TileContext(nc))
        consts = ctx.enter_context(tc.tile_pool(name="consts", bufs=1))
        xpool = ctx.enter_context(tc.tile_pool(name="xpool", bufs=5))
        tpool = ctx.enter_context(tc.tile_pool(name="tpool", bufs=4))
        hpool = ctx.enter_context(tc.tile_pool(name="hpool", bufs=4))
        opool = ctx.enter_context(tc.tile_pool(name="opool", bufs=3))
        ps1p = ctx.enter_context(tc.tile_pool(name="ps1p", bufs=2, space="PSUM"))
        ps2p = ctx.enter_context(tc.tile_pool(name="ps2p", bufs=2, space="PSUM"))
        ps3p = ctx.enter_context(tc.tile_pool(name="ps3p", bufs=2, space="PSUM"))

        w1_sb = consts.tile([128, 128], F16, tag="w1", name="w1_sb")
        nc.gpsimd.dma_start(out=w1_sb, in_=w1s[:])
        w2_sb = consts.tile([128, 128], F16, tag="w2", name="w2_sb")
        nc.gpsimd.dma_start(out=w2_sb, in_=w2s[:])
        w3_sb = consts.tile([128, 2048], F16, tag="w3", name="w3_sb")
        nc.gpsimd.dma_start(out=w3_sb, in_=w3s[:])
        bc1_sb = consts.tile([128, 1], F32, tag="bc1", name="bc1_sb")
        nc.gpsimd.dma_start(out=bc1_sb, in_=bc1[:])
        bc2_sb = consts.tile([128, 1], F32, tag="bc2", name="bc2_sb")
        nc.gpsimd.dma_start(out=bc2_sb, in_=bc2[:])
        bc3_sb = consts.tile([128, 1], F32, tag="bc3", name="bc3_sb")
        nc.gpsimd.dma_start(out=bc3_sb, in_=bc3[:])
        bm4_sb = consts.tile([128, 1], F32, tag="bm4", name="bm4_sb")
        nc.vector.memset(bm4_sb, -4.0)

        pools = (tpool, hpool)
        for pack in range(NPACK):
            ps3 = ps3p.tile([128, 512], F32, tag="ps3", name="ps3")
            for tau in range(16):
                t = pack * 16 + tau
                if tau % 2 == 0:
                    xt2 = xpool.tile([128, 2048], F16, tag="xt2", name="xt2")
                    nc.sync.dma_start(out=xt2, in_=xpk[t // 2])
                off = 1024 * (tau % 2)
                xh = xt2[:, off:off + 512]
                xl = xt2[:, off + 512:off + 1024]

                ps1 = ps1p.tile([128, 512], F32, tag="ps1", name="ps1")
                nc.tensor.matmul(ps1, w1_sb, xh, start=True, stop=False)
                nc.tensor.matmul(ps1, w1_sb, xl, start=False, stop=True)
                # every 8th layer-slot (2 per tile) takes the all-DVE form
                slot = 2 * t
                h1 = _layer_ep(nc, pools, ps1, bc1_sb, bm4_sb, F16,
                               dve_form=(slot % 8 == 7), tagp="l1")

                ps2 = ps2p.tile([128, 512], F32, tag="ps2", name="ps2")
                nc.tensor.matmul(ps2, w2_sb, h1, start=True, stop=True)
                h2 = _layer_ep(nc, pools, ps2, bc2_sb, bm4_sb, F16,
                               dve_form=((slot + 1) % 8 == 7), tagp="l2")

                nc.tensor.matmul(ps3, w3_sb[:, 128 * tau:128 * (tau + 1)], h2,
                                 start=(tau == 0), stop=(tau == 15))

            t3 = tpool.tile([128, 512], F32, tag="t3", name="t3")
            nc.vector.tensor_scalar(t3, ps3, bc3_sb, C2, ALU.add, ALU.add)
            y = tpool.tile([128, 512], F32, tag="y", name="y")
            nc.vector.tensor_scalar(y, t3, C2, None, ALU.subtract)
            oa = opool.tile([128, 512], F32, tag="oa", name="oa")
            nc.vector.tensor_scalar(oa, y, 1.0 / 256.0, None, ALU.mult)
            e = tpool.tile([128, 512], F32, tag="e", name="e")
            nc.scalar.activation(e, y, AF.Exp, bias=bm4_sb, scale=1.0 / 256.0)
            ob = opool.tile([128, 512], F32, tag="ob", name="ob")
            nc.vector.tensor_scalar(ob, e, float(np.exp(5.0)),
                                    float(np.exp(-4.6)), ALU.min, ALU.max)
            nc.sync.dma_start(out=outa[pack], in_=oa)
            nc.sync.dma_start(out=outb[pack], in_=ob)

    nc.compile()
    return nc


def _get_graph():
    if "nc" not in _compiled:
        _compiled["nc"] = _build_graph()
    return _compiled["nc"]


def _prep_weights(w0, b0, w1, b1, w_out, b_out):
    eye = np.eye(C, dtype=np.float32)
    W0s = (w0.T.astype(np.float32) + 256.0 * eye).astype(np.float16)
    W1s_small = ((w1.T.astype(np.float32) + 256.0 * eye) / 256.0).astype(np.float16)
    W3_small = (w_out.T.astype(np.float32) / 256.0).astype(np.float16)  # [32, 2]

    w1s = np.zeros((128, 128), np.float16)
    w2s = np.zeros((128, 128), np.float16)
    for b in range(4):
        w1s[32 * b:32 * b + 32, 32 * b:32 * b + 32] = W0s
        w2s[32 * b:32 * b + 32, 32 * b:32 * b + 32] = W1s_small

    # mm3 stationary for inner-loop index tau: out partition m = 8 tau + 4 o + b
    w3pack = np.zeros((16, 128, 128), np.float16)
    for tau in range(16):
        for b in range(4):
            for o in range(2):
                w3pack[tau, 32 * b:32 * b + 32, 8 * tau + 4 * o + b] = W3_small[:, o]
    w3s = np.ascontiguousarray(w3pack.transpose(1, 0, 2).reshape(128, 2048))

    bc1 = np.zeros((128, 1), np.float32)
    bc2 = np.zeros((128, 1), np.float32)
    bc3 = np.zeros((128, 1), np.float32)
    for b in range(4):
        bc1[32 * b:32 * b + 32, 0] = b0.astype(np.float32) / 256.0 + CTIE
        bc2[32 * b:32 * b + 32, 0] = b1.astype(np.float32) / 256.0 + CTIE
    for tau in range(16):
        for o in range(2):
            for b in range(4):
                bc3[8 * tau + 4 * o + b, 0] = float(b_out[o]) / 256.0 + CTIE
    return w1s, w2s, w3s, bc1, bc2, bc3


def _prep_x_core(xs):
    """[S, 32] f32 -> xpk [NT//2, 128, 2048] fp16 (hi/lo, 2 tiles per row)."""
    xd = xs.reshape(NT, 4, 512, C).transpose(0, 1, 3, 2).reshape(NT, 128, 512)
    xh = xd.astype(np.float16)
    xl = (xd - xh.astype(np.float32)).astype(np.float16)
    xpk = np.empty((NT // 2, 128, 2048), np.float16)
    xpk[:, :, 0:512] = xh[0::2]
    xpk[:, :, 512:1024] = xl[0::2]
    xpk[:, :, 1024:1536] = xh[1::2]
    xpk[:, :, 1536:2048] = xl[1::2]
    return xpk


def kernel(x, w0, b0, w1, b1, w_out, b_out):
    x = np.ascontiguousarray(np.asarray(x, np.float32))
    w1s, w2s, w3s, bc1, bc2, bc3 = _prep_weights(
        np.asarray(w0), np.asarray(b0), np.asarray(w1), np.asarray(b1),
        np.asarray(w_out), np.asarray(b_out))

    nc = _get_graph()

    in_maps = []
    for i in range(NCORES):
        xpk = _prep_x_core(x[i * S:(i + 1) * S])
        in_maps.append({"xpk": xpk, "w1s": w1s, "w2s": w2s,
                        "w3s": w3s, "bc1": bc1, "bc2": bc2, "bc3": bc3})

    res = run_bass_kernel_spmd(nc, in_maps, list(range(NCORES))).results

    mu = np.empty(B, np.float32)
    ls = np.empty(B, np.float32)
    sc = np.empty(B, np.float32)
    for i in range(NCORES):
        # outa[pack, 8 tau + 4 o + b, f'] = raw(row = 2048(16 pack+tau)+512 b+f', o)
        a = np.asarray(res[i]["outa"]).reshape(NPACK, 16, 2, 4, 512)
        bb = np.asarray(res[i]["outb"]).reshape(NPACK, 16, 2, 4, 512)
        sl = slice(i * S, (i + 1) * S)
        mu[sl] = a[:, :, 0].reshape(S)
        ls[sl] = a[:, :, 1].reshape(S)
        sc[sl] = bb[:, :, 1].reshape(S)
    return mu, sc, ls


if __name__ == "__main__":
    rng = np.random.default_rng(0)
    x = rng.standard_normal((B, C)).astype(np.float32)
    w0 = np.round(rng.standard_normal((C, C)) * 13).astype(np.float32)
    b0 = np.round(rng.standard_normal(C) * 3000).astype(np.float32)
    w1 = np.round(rng.standard_normal((C, C)) * 13).astype(np.float32)
    b1 = np.round(rng.standard_normal(C) * 3000).astype(np.float32)
    w_out = np.round(rng.standard_normal((2, C)) * 13).astype(np.float32)
    b_out = np.round(rng.standard_normal(2) * 3000).astype(np.float32)
    out = kernel(x, w0, b0, w1, b1, w_out, b_out)
    print([o.shape for o in out], [float(np.abs(o).mean()) for o in out])


# revision 10
# speedup vs baseline: 32157.6412x; 1.0120x over previous
"""Trainium2 Bass kernel for nn_ArmInt_19911468384433 (dense_mlp, 8 cores).

Data-parallel: x [2097152, 32] sharded by rows across 8 NeuronCores; tiny
32x32 weights folded/replicated. All math (3 integer-MLP layers with
emulated fixed-point rounding, exp/clip head) runs on device. Host does
layout permutation (shard + transpose-pack of input, fp16 hi/lo split,
inverse reshape of output) and weight folding only.

Device algorithm per core (S = 262144 rows = 128 tiles of 2048 rows):
  x pre-packed on host as xd*[t, 32b+c, f'] = x[2048 t + 512 b + f', c],
  split x = xhi + xlo (both fp16; 22-bit combined mantissa, exact enough:
  boundary-flip rate below the fp32 accumulation-order noise floor).
  Per tile:
    mm1 : ps1 = W1s.T @ xhi + W1s.T @ xlo   (fp16, PSUM f32 accumulate)
    ep1 : t = Relu(ps1 + bc1) (ACT, bias AP)  then h1 = rne(t) via DVE
          fused ts (+2^23, -2^23) -> fp16   [or the all-DVE form below]
    mm2 : ps2 = W2s.T @ h1  (fp16);  ep2 likewise -> h2
    mm3 : ps3pack += W3s_tau.T @ h2  (16 tiles accumulate into one bank)
  Every 4th layer-slot uses the all-DVE form to balance engines:
    u = ts(ps + bc, + 2^23) ; h = ts(u - 2^23, max 0) -> fp16
  Per 16-tile pack (DVE except Exp):
    t3 = ps3pack + bc3 ; y = rne(t3) ; outA = y/256
    e = Exp(y/256 - 4) (ACT) ; outB = clip(e, e^-4.6, e^5)

rne(v + 2^-9) == trunc-round-half-away-from-zero on the 1/256-granular
values here; verified vs reference at rel err ~5e-3 (fp32 order noise).
"""
import sys

sys.path.insert(0, "/opt/trn_rl_repo")

from contextlib import ExitStack

import numpy as np

import concourse.bacc as bacc
import concourse.bass as bass
import concourse.tile as tile
from concourse import mybir
from concourse.bass_utils import run_bass_kernel_spmd

F32 = mybir.dt.float32
F16 = mybir.dt.float16
AF = mybir.ActivationFunctionType
ALU = mybir.AluOpType

B = 2097152
C = 32
NCORES = 8
S = B // NCORES            # 262144 rows per core
NT = S // 2048             # 128 tiles per core
NPACK = NT // 16           # 8 packs per core
C2 = float(2.0 ** 23)
CTIE = float(2.0 ** -9)

_compiled = {}


def _layer_ep(nc, pools, ps, bc_sb, bcm_sb, out_dt, dve_form, tagp):
    """PSUM -> relu(rne(v + bc)) -> SBUF tile (out_dt)."""
    tpool, hpool = pools
    if not dve_form:
        t = tpool.tile([128, 512], F32, tag=tagp + "t", name="t")
        nc.scalar.activation(t, ps, AF.Relu, bias=bc_sb, scale=1.0)
        h = hpool.tile([128, 512], out_dt, tag=tagp + "h", name="h")
        nc.vector.tensor_scalar(h, t, C2, C2, ALU.add, ALU.subtract)
    else:
        u = tpool.tile([128, 512], F32, tag=tagp + "t", name="u")
        nc.vector.tensor_scalar(u, ps, bc_sb, C2, ALU.add, ALU.add)
        h = hpool.tile([128, 512], out_dt, tag=tagp + "h", name="h")
        nc.vector.tensor_scalar(h, u, C2, 0.0, ALU.subtract, ALU.max)
    return h


def _build_graph():
    nc = bacc.Bacc("TRN2", target_bir_lowering=False, debug=False)
    # hi/lo fp16 halves for two consecutive tiles packed in one 512KB DMA:
    # [:, 0:512]=hi(2j) [:, 512:1024]=lo(2j) [:, 1024:1536]=hi(2j+1) ...
    xpk = nc.declare_dram_parameter("xpk", [NT // 2, 128, 2048], F16, isOutput=False)
    w1s = nc.declare_dram_parameter("w1s", [128, 128], F16, isOutput=False)
    w2s = nc.declare_dram_parameter("w2s", [128, 128], F16, isOutput=False)
    w3s = nc.declare_dram_parameter("w3s", [128, 2048], F16, isOutput=False)
    bc1 = nc.declare_dram_parameter("bc1", [128, 1], F32, isOutput=False)
    bc2 = nc.declare_dram_parameter("bc2", [128, 1], F32, isOutput=False)
    bc3 = nc.declare_dram_parameter("bc3", [128, 1], F32, isOutput=False)
    outa = nc.declare_dram_parameter("outa", [NPACK, 128, 512], F16, isOutput=True)
    outb = nc.declare_dram_parameter("outb", [NPACK, 128, 512], F16, isOutput=True)

    with ExitStack() as ctx:
        tc = ctx.enter_context(tile.TileContext(nc))
        consts = ctx.enter_context(tc.tile_pool(name="consts", bufs=1))
        xpool = ctx.enter_context(tc.tile_pool(name="xpool", bufs=5))
        tpool = ctx.enter_context(tc.tile_pool(name="tpool", bufs=4))
        hpool = ctx.enter_context(tc.tile_pool(name="hpool", bufs=4))
        opool = ctx.enter_context(tc.tile_pool(name="opool", bufs=3))
        ps1p = ctx.enter_context(tc.tile_pool(name="ps1p", bufs=2, space="PSUM"))
        ps2p = ctx.enter_context(tc.tile_pool(name="ps2p", bufs=2, space="PSUM"))
        ps3p = ctx.enter_context(tc.tile_pool(name="ps3p", bufs=2, space="PSUM"))

        w1_sb = consts.tile([128, 128], F16, tag="w1", name="w1_sb")
        nc.gpsimd.dma_start(out=w1_sb, in_=w1s[:])
        w2_sb = consts.tile([128, 128], F16, tag="w2", name="w2_sb")
        nc.gpsimd.dma_start(out=w2_sb, in_=w2s[:])
        w3_sb = consts.tile([128, 2048], F16, tag="w3", name="w3_sb")
        nc.gpsimd.dma_start(out=w3_sb, in_=w3s[:])
        bc1_sb = consts.tile([128, 1], F32, tag="bc1", name="bc1_sb")
        nc.gpsimd.dma_start(out=bc1_sb, in_=bc1[:])
        bc2_sb = consts.tile([128, 1], F32, tag="bc2", name="bc2_sb")
        nc.gpsimd.dma_start(out=bc2_sb, in_=bc2[:])
        bc3_sb = consts.tile([128, 1], F32, tag="bc3", name="bc3_sb")
        nc.gpsimd.dma_start(out=bc3_sb, in_=bc3[:])
        bm4_sb = consts.tile([128, 1], F32, tag="bm4", name="bm4_sb")
        nc.vector.memset(bm4_sb, -4.0)

        pools = (tpool, hpool)
        for pack in range(NPACK):
            ps3 = ps3p.tile([128, 512], F32, tag="ps3", name="ps3")
            for tau in range(16):
                t = pack * 16 + tau
                if tau % 2 == 0:
                    xt2 = xpool.tile([128, 2048], F16, tag="xt2", name="xt2")
                    deng = nc.sync if (t // 2) % 2 == 0 else nc.gpsimd
                    deng.dma_start(out=xt2, in_=xpk[t // 2])
                off = 1024 * (tau % 2)
                xh = xt2[:, off:off + 512]
                xl = xt2[:, off + 512:off + 1024]

                ps1 = ps1p.tile([128, 512], F32, tag="ps1", name="ps1")
                nc.tensor.matmul(ps1, w1_sb, xh, start=True, stop=False)
                nc.tensor.matmul(ps1, w1_sb, xl, start=False, stop=True)
                # every 8th layer-slot (2 per tile) takes the all-DVE form
                slot = 2 * t
                h1 = _layer_ep(nc, pools, ps1, bc1_sb, bm4_sb, F16,
                               dve_form=(slot % 8 == 7), tagp="l1")

                ps2 = ps2p.tile([128, 512], F32, tag="ps2", name="ps2")
                nc.tensor.matmul(ps2, w2_sb, h1, start=True, stop=True)
                h2 = _layer_ep(nc, pools, ps2, bc2_sb, bm4_sb, F16,
                               dve_form=((slot + 1) % 8 == 7), tagp="l2")

                nc.tensor.matmul(ps3, w3_sb[:, 128 * tau:128 * (tau + 1)], h2,
                                 start=(tau == 0), stop=(tau == 15))

            t3 = tpool.tile([128, 512], F32, tag="t3", name="t3")
            nc.vector.tensor_scalar(t3, ps3, bc3_sb, C2, ALU.add, ALU.add)
            y = tpool.tile([128, 512], F32, tag="y", name="y")
            nc.vector.tensor_scalar(y, t3, C2, None, ALU.subtract)
            oa = opool.tile([128, 512], F16, tag="oa", name="oa")
            nc.vector.tensor_scalar(oa, y, 1.0 / 256.0, None, ALU.mult)
            e = tpool.tile([128, 512], F32, tag="e", name="e")
            nc.scalar.activation(e, y, AF.Exp, bias=bm4_sb, scale=1.0 / 256.0)
            ob = opool.tile([128, 512], F16, tag="ob", name="ob")
            nc.vector.tensor_scalar(ob, e, float(np.exp(5.0)),
                                    float(np.exp(-4.6)), ALU.min, ALU.max)
            nc.sync.dma_start(out=outa[pack], in_=oa)
            nc.sync.dma_start(out=outb[pack], in_=ob)

    nc.compile()
    return nc


def _get_graph():
    if "nc" not in _compiled:
        _compiled["nc"] = _build_graph()
    return _compiled["nc"]


def _prep_weights(w0, b0, w1, b1, w_out, b_out):
    eye = np.eye(C, dtype=np.float32)
    W0s = (w0.T.astype(np.float32) + 256.0 * eye).astype(np.float16)
    W1s_small = ((w1.T.astype(np.float32) + 256.0 * eye) / 256.0).astype(np.float16)
    W3_small = (w_out.T.astype(np.float32) / 256.0).astype(np.float16)  # [32, 2]

    w1s = np.zeros((128, 128), np.float16)
    w2s = np.zeros((128, 128), np.float16)
    for b in range(4):
        w1s[32 * b:32 * b + 32, 32 * b:32 * b + 32] = W0s
        w2s[32 * b:32 * b + 32, 32 * b:32 * b + 32] = W1s_small

    # mm3 stationary for inner-loop index tau: out partition m = 8 tau + 4 o + b
    w3pack = np.zeros((16, 128, 128), np.float16)
    for tau in range(16):
        for b in range(4):
            for o in range(2):
                w3pack[tau, 32 * b:32 * b + 32, 8 * tau + 4 * o + b] = W3_small[:, o]
    w3s = np.ascontiguousarray(w3pack.transpose(1, 0, 2).reshape(128, 2048))

    bc1 = np.zeros((128, 1), np.float32)
    bc2 = np.zeros((128, 1), np.float32)
    bc3 = np.zeros((128, 1), np.float32)
    for b in range(4):
        bc1[32 * b:32 * b + 32, 0] = b0.astype(np.float32) / 256.0 + CTIE
        bc2[32 * b:32 * b + 32, 0] = b1.astype(np.float32) / 256.0 + CTIE
    for tau in range(16):
        for o in range(2):
            for b in range(4):
                bc3[8 * tau + 4 * o + b, 0] = float(b_out[o]) / 256.0 + CTIE
    return w1s, w2s, w3s, bc1, bc2, bc3


def _prep_x_core(xs):
    """[S, 32] f32 -> xpk [NT//2, 128, 2048] fp16 (hi/lo, 2 tiles per row)."""
    xd = xs.reshape(NT, 4, 512, C).transpose(0, 1, 3, 2).reshape(NT, 128, 512)
    xh = xd.astype(np.float16)
    xl = (xd - xh.astype(np.float32)).astype(np.float16)
    xpk = np.empty((NT // 2, 128, 2048), np.float16)
    xpk[:, :, 0:512] = xh[0::2]
    xpk[:, :, 512:1024] = xl[0::2]
    xpk[:, :, 1024:1536] = xh[1::2]
    xpk[:, :, 1536:2048] = xl[1::2]
    return xpk


def kernel(x, w0, b0, w1, b1, w_out, b_out):
    x = np.ascontiguousarray(np.asarray(x, np.float32))
    w1s, w2s, w3s, bc1, bc2, bc3 = _prep_weights(
        np.asarray(w0), np.asarray(b0), np.asarray(w1), np.asarray(b1),
        np.asarray(w_out), np.asarray(b_out))

    nc = _get_graph()

    in_maps = []
    for i in range(NCORES):
        xpk = _prep_x_core(x[i * S:(i + 1) * S])
        in_maps.append({"xpk": xpk, "w1s": w1s, "w2s": w2s,
                        "w3s": w3s, "bc1": bc1, "bc2": bc2, "bc3": bc3})

    res = run_bass_kernel_spmd(nc, in_maps, list(range(NCORES))).results

    mu = np.empty(B, np.float32)
    ls = np.empty(B, np.float32)
    sc = np.empty(B, np.float32)
    for i in range(NCORES):
        # outa[pack, 8 tau + 4 o + b, f'] = raw(row = 2048(16 pack+tau)+512 b+f', o)
        a = np.asarray(res[i]["outa"], np.float32).reshape(NPACK, 16, 2, 4, 512)
        bb = np.asarray(res[i]["outb"], np.float32).reshape(NPACK, 16, 2, 4, 512)
        sl = slice(i * S, (i + 1) * S)
        mu[sl] = a[:, :, 0].reshape(S)
        ls[sl] = a[:, :, 1].reshape(S)
        sc[sl] = bb[:, :, 1].reshape(S)
    return mu, sc, ls


if __name__ == "__main__":
    rng = np.random.default_rng(0)
    x = rng.standard_normal((B, C)).astype(np.float32)
    w0 = np.round(rng.standard_normal((C, C)) * 13).astype(np.float32)
    b0 = np.round(rng.standard_normal(C) * 3000).astype(np.float32)
    w1 = np.round(rng.standard_normal((C, C)) * 13).astype(np.float32)
    b1 = np.round(rng.standard_normal(C) * 3000).astype(np.float32)
    w_out = np.round(rng.standard_normal((2, C)) * 13).astype(np.float32)
    b_out = np.round(rng.standard_normal(2) * 3000).astype(np.float32)
    out = kernel(x, w0, b0, w1, b1, w_out, b_out)
    print([o.shape for o in out], [float(np.abs(o).mean()) for o in out])
